# revision 1
# baseline (speedup 1.0000x reference)
"""Trainium2 Bass kernel for the Jastrow-factor nn.Module.

Math (per walker w):
  EN: r_en[w,e,n] = |x_we - nuc_n|
      J_en   = sum_{e,n} -q_n * r/(1+softplus(b_en_n)*r)
      J_ennn = s_en * sum_e MLP8(r_en[w,e,:]**2)        (8->32->32->1, silu)
  EE: r_ee[w,p] over 496 unordered pairs p=(i,j)
      J_ee   = sum_p a_p * r/(1+softplus(b_ee)*r)
      J_eenn = s_ee * sum_p MLP1(r_ee[w,p])             (1->32->32->1, silu)
  out[w] = J_en + J_ennn + J_ee + J_eenn

Distribution: pure data parallel, 1024 walkers per core on 8 cores.

Device layout strategy per core (W=1024 walkers):
  EN: one augmented matmul lhsT[20,32] @ x20[20, 8192] produces r_en^2 for
      4 electron-groups x 8 nuclei stacked on K; the MLP runs as
      block-diagonal matmuls in [feature, batch] layout; the L3 + classical
      charge-weighted sums accumulate in PSUM rows; a segmented reduce over
      the 8 electrons of each group column yields [1, 1024].
  EE: pair distances via 31 diagonal-offset subtractions in
      [128 walker-partitions, free] layout (full-lane DVE), one big ACT
      sqrt, PE transposes into 4 tiles [124 pairs, 1024 walkers], then the
      MLP with per-group row-selection weight matrices (K=124) so every
      matmul operand sits at partition base 0.  Layer-3 and the classical
      term accumulate into one PSUM row; J_ee falls out of PSUM directly.
"""

import numpy as np

N_CORES = 8
N_W, N_E, N_NUC, D_H = 8192, 32, 8, 32
WC = N_W // N_CORES          # walkers per core
NT = WC // 128               # walker tiles per core (8)
P_PAIRS = N_E * (N_E - 1) // 2   # 496
NB = 4                       # rT pair tiles, 124 pairs each
PB = P_PAIRS // NB           # 124
NSEL = PB // 4               # 31 selection matrices
NQEN = WC * 8 // 512         # 16 EN column chunks of 512


def _pair_list():
    ps = []
    for d in range(1, N_E):
        for e in range(N_E - d):
            ps.append((e, e + d))
    return ps


_PAIRS = _pair_list()
assert len(_PAIRS) == P_PAIRS


def _softplus(x):
    return np.log1p(np.exp(-np.abs(x))) + np.maximum(x, 0.0)


# ----------------------------------------------------------------------------
# device program
# ----------------------------------------------------------------------------

_CACHE = {}


def _build_program():
    from contextlib import ExitStack

    import concourse.bacc as bacc
    import concourse.bass as bass
    import concourse.tile as tile
    from concourse import mybir

    f32 = mybir.dt.float32
    AF = mybir.ActivationFunctionType
    ALU = mybir.AluOpType
    AX = mybir.AxisListType

    nc = bacc.Bacc()

    def din(name, shape):
        return nc.declare_dram_parameter(name, list(shape), f32, isOutput=False)

    # per-core data
    d_xwp = din("xwp", [128, NT, 96])            # walker-partition coords
    d_x20 = din("x20", [128, 2048])              # EN augmented rhs, 32-aligned
    # shared weights / constants
    d_ident = din("ident", [128, 128])
    d_wendist = din("wendist", [128, 32])
    d_wenl1 = din("wenl1", [128, 128])           # 4x vstack of blockdiag4(W1_en)
    d_wenl2 = din("wenl2", [128, 128])
    d_vecs = din("vecs", [128, 16])
    d_weesel = din("weesel", [PB, NSEL * 128])   # 31 selection matrices
    d_weel2 = din("weel2", [128, 128])
    d_out = nc.declare_dram_parameter("out", [1, WC], f32, isOutput=True)

    MM = nc.tensor.matmul

    with ExitStack() as top:
        tc = top.enter_context(tile.TileContext(nc))
        const = top.enter_context(tc.tile_pool(name="const", bufs=1))
        work = top.enter_context(tc.tile_pool(name="work", bufs=1))

        def load(dram, shape):
            t = const.tile(shape, f32, name=dram.name, tag=dram.name)
            nc.gpsimd.dma_start(out=t[:], in_=dram[:])
            return t

        xwp = load(d_xwp, [128, NT, 96])
        x20 = load(d_x20, [128, 2048])
        ident = load(d_ident, [128, 128])
        wendist = load(d_wendist, [128, 32])
        wenl1 = load(d_wenl1, [128, 128])
        wenl2 = load(d_wenl2, [128, 128])
        vecs = load(d_vecs, [128, 16])
        weesel = load(d_weesel, [PB, NSEL, 128])
        weel2 = load(d_weel2, [128, 128])
        wenl3 = vecs[:, 0:1]
        wencls = vecs[:, 1:2]
        b1en = vecs[:, 2:3]
        b2en = vecs[:, 3:4]
        bensp = vecs[:, 4:5]
        weel3 = vecs[:, 5:6]
        b1ee = vecs[:, 6:7]
        b2ee = vecs[:, 7:8]
        beesp = vecs[:, 8:9]
        cconst = vecs[0:1, 13:14]

        # ------------------------------------------------------------------
        # EE distances in walker-partition layout
        # r2wp[p, t, col] ; col = pair index by diagonal order, padded to 512
        # ------------------------------------------------------------------
        r2wp = work.tile([128, NT, 512], f32)
        nc.vector.memset(r2wp[:], 0.0)
        dpool_cm = tc.tile_pool(name="dpool", bufs=2)
        dpool = dpool_cm.__enter__()
        off = 0
        for d in range(1, N_E):
            L = N_E - d
            dd = dpool.tile([128, NT, 96], f32, tag="dd")
            sq = dpool.tile([128, NT, 96], f32, tag="sq")
            nc.vector.tensor_sub(
                dd[:, :, : 3 * L], xwp[:, :, : 3 * L], xwp[:, :, 3 * d :]
            )
            nc.vector.tensor_mul(
                sq[:, :, : 3 * L], dd[:, :, : 3 * L], dd[:, :, : 3 * L]
            )
            sq3 = sq[:, :, : 3 * L].rearrange("p t (e c) -> p c t e", c=3)
            nc.vector.tensor_add(r2wp[:, :, off : off + L], sq3[:, 0], sq3[:, 1])
            nc.vector.tensor_add(
                r2wp[:, :, off : off + L], r2wp[:, :, off : off + L], sq3[:, 2]
            )
            off += L
        assert off == P_PAIRS
        dpool_cm.__exit__(None, None, None)

        # one big sqrt (ACT, Sqrt table set), in place: rwp aliases r2wp
        rwp = r2wp
        nc.scalar.sqrt(rwp[:], r2wp[:])

        # ------------------------------------------------------------------
        # EN r^2 via augmented matmul, packed 4 chunks deep on partitions
        # ------------------------------------------------------------------
        r2en = work.tile([128, 4, 512], f32)   # partition (c,g,n), free (qq,512)
        with tc.tile_pool(name="edps", bufs=2, space=bass.MemorySpace.PSUM) as edps:
            for qq in range(4):
                ps = edps.tile([128, 512], f32)
                for c in range(4):
                    MM(
                        ps[32 * c : 32 * c + 32, :],
                        wendist[32 * c : 32 * c + 20, :],
                        x20[32 * c : 32 * c + 20, 512 * qq : 512 * qq + 512],
                        start=True,
                        stop=True,
                        tile_position=(32 * c, 32 * c),
                    )
                nc.vector.tensor_copy(r2en[:, qq, :], ps[:])

        ren = work.tile([128, 4, 512], f32)
        nc.scalar.sqrt(ren[:], r2en[:])

        # EN classical: t = r / (1 + softplus(b_en)*r)
        uen = work.tile([128, 4, 512], f32)
        nc.vector.tensor_scalar(
            uen[:], ren[:], bensp, 1.0, op0=ALU.mult, op1=ALU.add
        )
        nc.vector.reciprocal_approx_fast(out=uen[:], in_=uen[:])
        ten = ren
        nc.vector.tensor_mul(ten[:], ren[:], uen[:])

        # ------------------------------------------------------------------
        # EN MLP + classical reduction -> jen[1, 1024]
        # ------------------------------------------------------------------
        jen = work.tile([1, WC], f32)
        with (
            tc.tile_pool(name="enps1", bufs=2, space=bass.MemorySpace.PSUM) as enps1,
            tc.tile_pool(name="enps2", bufs=1, space=bass.MemorySpace.PSUM) as enps2,
            tc.tile_pool(name="enjen", bufs=2, space=bass.MemorySpace.PSUM) as enjen,
            tc.tile_pool(name="enh", bufs=2) as enh,
        ):
            for bidx in range(NQEN // 2):
                qs = [2 * bidx, 2 * bidx + 1]
                ps1 = enps1.tile([128, 2, 512], f32, tag="ps1")
                for i, q in enumerate(qs):
                    c, qq = q % 4, q // 4
                    MM(
                        ps1[:, i, :],
                        wenl1[32 * c : 32 * c + 32, :],
                        r2en[32 * c : 32 * c + 32, qq, :],
                        start=True,
                        stop=True,
                        tile_position=(32 * c, 0),
                    )
                h1 = enh.tile([128, 2, 512], f32, tag="h1")
                nc.scalar.activation(h1[:], ps1[:], AF.Silu, bias=b1en)
                ps2 = enps2.tile([128, 2, 512], f32, tag="ps2")
                for i in range(2):
                    MM(ps2[:, i, :], wenl2[:], h1[:, i, :], start=True, stop=True)
                h2 = enh.tile([128, 2, 512], f32, tag="h2")
                nc.scalar.activation(h2[:], ps2[:], AF.Silu, bias=b2en)
                for i, q in enumerate(qs):
                    c, qq = q % 4, q // 4
                    jt = enjen.tile([1, 512], f32, tag="jt")
                    MM(
                        jt[0:1, :],
                        wencls[32 * c : 32 * c + 32],
                        ten[32 * c : 32 * c + 32, qq, :],
                        start=True,
                        stop=False,
                        skip_group_check=True,
                        tile_position=(32 * c, 0),
                    )
                    MM(
                        jt[0:1, :],
                        wenl3,
                        h2[:, i, :],
                        start=False,
                        stop=True,
                        skip_group_check=True,
                    )
                    nc.vector.reduce_sum(
                        jen[0:1, 64 * q : 64 * q + 64],
                        jt[0:1, :].rearrange("p (w e) -> p w e", e=8),
                        axis=AX.X,
                    )

        # ------------------------------------------------------------------
        # EE transposes: rwp -> rT[b] [124 pairs, 1024 walkers]
        # ------------------------------------------------------------------
        rT = [work.tile([PB, WC], f32, tag=f"rT{b}", name=f"rT{b}") for b in range(NB)]
        with tc.tile_pool(name="ptps", bufs=3, space=bass.MemorySpace.PSUM) as ptps:
            for t in range(NT):
                for b in range(NB):
                    pt = ptps.tile([PB, 128], f32, tag="pt")
                    nc.tensor.transpose(
                        pt[:], rwp[:, t, PB * b : PB * b + PB], ident[:]
                    )
                    nc.vector.tensor_copy(rT[b][:, 128 * t : 128 * t + 128], pt[:])

        # ------------------------------------------------------------------
        # EE classical + MLP, accumulating into jee[1, 1024] (PSUM)
        # ------------------------------------------------------------------
        with (
            tc.tile_pool(name="jeeps", bufs=1, space=bass.MemorySpace.PSUM) as jeeps,
            tc.tile_pool(name="eecls", bufs=2) as eecls,
        ):
            jee = jeeps.tile([1, WC], f32)
            for b in range(NB):
                u = eecls.tile([PB, WC], f32, tag="u")
                nc.vector.tensor_scalar(
                    u[:], rT[b][:], beesp[0:PB], 1.0, op0=ALU.mult, op1=ALU.add
                )
                nc.vector.reciprocal_approx_fast(out=u[:], in_=u[:])
                t_ee = eecls.tile([PB, WC], f32, tag="t")
                nc.vector.tensor_mul(t_ee[:], rT[b][:], u[:])
                for h in range(2):
                    MM(
                        jee[0:1, 512 * h : 512 * h + 512],
                        vecs[0:PB, 9 + b : 10 + b],
                        t_ee[:, 512 * h : 512 * h + 512],
                        start=(b == 0),
                        stop=False,
                        skip_group_check=True,
                    )

            with (
                tc.tile_pool(
                    name="eeps1", bufs=2, space=bass.MemorySpace.PSUM
                ) as eeps1,
                tc.tile_pool(
                    name="eeps2", bufs=1, space=bass.MemorySpace.PSUM
                ) as eeps2,
                tc.tile_pool(name="eeh", bufs=2) as eeh,
            ):
                for q in range(PB):
                    b, m = divmod(q, NSEL)
                    ps1 = eeps1.tile([128, 2, 512], f32, tag="ps1")
                    for h in range(2):
                        MM(
                            ps1[:, h, :],
                            weesel[:, m, :],
                            rT[b][:, 512 * h : 512 * h + 512],
                            start=True,
                            stop=True,
                        )
                    h1 = eeh.tile([128, 2, 512], f32, tag="h1")
                    nc.scalar.activation(h1[:], ps1[:], AF.Silu, bias=b1ee)
                    ps2 = eeps2.tile([128, 2, 512], f32, tag="ps2")
                    for h in range(2):
                        MM(ps2[:, h, :], weel2[:], h1[:, h, :], start=True, stop=True)
                    h2 = eeh.tile([128, 2, 512], f32, tag="h2")
                    nc.scalar.activation(h2[:], ps2[:], AF.Silu, bias=b2ee)
                    last = q == PB - 1
                    for h in range(2):
                        MM(
                            jee[0:1, 512 * h : 512 * h + 512],
                            weel3,
                            h2[:, h, :],
                            start=False,
                            stop=last,
                            skip_group_check=True,
                        )

            # final: out = (jee + C) + jen
            out_sb = work.tile([1, WC], f32)
            nc.vector.scalar_tensor_tensor(
                out=out_sb[:],
                in0=jee[:],
                scalar=cconst,
                in1=jen[:],
                op0=ALU.add,
                op1=ALU.add,
            )
            nc.gpsimd.dma_start(out=d_out[:], in_=out_sb[:])

    nc.finalize()
    return nc


def _get_program():
    if "nc" not in _CACHE:
        _CACHE["nc"] = _build_program()
    return _CACHE["nc"]


# ----------------------------------------------------------------------------
# host-side input prep
# ----------------------------------------------------------------------------


def _shared_inputs(r_nuclei, charges, spin_mask_parallel, b_en, b_ee,
                   W1_en, b1_en, W2_en, b2_en, W3_en, b3_en,
                   W1_ee, b1_ee, W2_ee, b2_ee, W3_ee, b3_ee,
                   scale_en, scale_ee):
    f = np.float32
    nuc = np.asarray(r_nuclei, f)
    q = np.asarray(charges, f)
    sm = np.asarray(spin_mask_parallel)
    s_en = float(np.asarray(scale_en))
    s_ee = float(np.asarray(scale_ee))

    out = {}
    out["ident"] = np.eye(128, dtype=f)

    # EN distance matmul weights [20, 32], vstacked 4x to [128, 32]
    wd = np.zeros((20, 32), f)
    qn = (nuc ** 2).sum(-1)
    for g in range(4):
        for n in range(N_NUC):
            col = g * 8 + n
            wd[g * 5 : g * 5 + 3, col] = -2.0 * nuc[n]
            wd[g * 5 + 3, col] = 1.0
            wd[g * 5 + 4, col] = qn[n]
    wd4 = np.zeros((128, 32), f)
    for c in range(4):
        wd4[32 * c : 32 * c + 20] = wd
    out["wendist"] = wd4

    W1e, W2e, W3e = np.asarray(W1_en, f), np.asarray(W2_en, f), np.asarray(W3_en, f)
    l1bd = np.zeros((32, 128), f)
    l2bd = np.zeros((128, 128), f)
    for g in range(4):
        l1bd[g * 8 : g * 8 + 8, g * 32 : g * 32 + 32] = W1e
        l2bd[g * 32 : g * 32 + 32, g * 32 : g * 32 + 32] = W2e
    out["wenl1"] = np.tile(l1bd, (4, 1))
    out["wenl2"] = l2bd
    vecs = np.zeros((128, 16), f)
    vecs[:, 0] = np.tile(s_en * W3e.reshape(32), 4)
    vecs[:, 1] = np.tile(-q, 16)
    vecs[:, 2] = np.tile(np.asarray(b1_en, f).reshape(32), 4)
    vecs[:, 3] = np.tile(np.asarray(b2_en, f).reshape(32), 4)
    vecs[:, 4] = np.tile(_softplus(np.asarray(b_en, f)).reshape(8), 16)

    W1p, W2p, W3p = np.asarray(W1_ee, f), np.asarray(W2_ee, f), np.asarray(W3_ee, f)
    sel = np.zeros((NSEL, PB, 128), f)
    for m in range(NSEL):
        for j in range(4):
            sel[m, 4 * m + j, j * 32 : j * 32 + 32] = W1p[0]
    out["weesel"] = np.ascontiguousarray(sel.transpose(1, 0, 2)).reshape(
        PB, NSEL * 128
    )
    l2ee = np.zeros((128, 128), f)
    for j in range(4):
        l2ee[j * 32 : j * 32 + 32, j * 32 : j * 32 + 32] = W2p
    out["weel2"] = l2ee
    vecs[:, 5] = np.tile(s_ee * W3p.reshape(32), 4)
    vecs[:, 6] = np.tile(np.asarray(b1_ee, f).reshape(32), 4)
    vecs[:, 7] = np.tile(np.asarray(b2_ee, f).reshape(32), 4)
    vecs[:, 8] = float(_softplus(np.asarray(b_ee, f).reshape(1))[0])

    a_all = np.empty((P_PAIRS,), f)
    for p, (i, j) in enumerate(_PAIRS):
        a_all[p] = 0.25 if sm[i, j] else 0.5
    vecs[0:PB, 9:13] = a_all.reshape(NB, PB).T

    vecs[0, 13] = N_E * s_en * float(np.asarray(b3_en).reshape(-1)[0]) + \
        P_PAIRS * s_ee * float(np.asarray(b3_ee).reshape(-1)[0])
    out["vecs"] = vecs
    return out


def _core_inputs(xs):
    """Per-core tensors from the walker shard xs [WC, 32, 3]."""
    f = np.float32
    xs = np.asarray(xs, f)
    xwp = np.ascontiguousarray(
        xs.reshape(NT, 128, 96).transpose(1, 0, 2)
    )  # [128, NT, 96]
    s = (xs ** 2).sum(-1)  # [WC, 32]
    x20f = np.empty((20, WC * 8), f)
    for g in range(4):
        blk = np.empty((5, WC, 8), f)
        blk[0:3] = xs[:, g * 8 : (g + 1) * 8, :].transpose(2, 0, 1)
        blk[3] = s[:, g * 8 : (g + 1) * 8]
        blk[4] = 1.0
        x20f[g * 5 : (g + 1) * 5] = blk.reshape(5, WC * 8)
    # pack 16 column-chunks as [qq cols x c partition-slots], 32-aligned
    x20 = np.zeros((128, 2048), f)
    for q in range(16):
        c, qq = q % 4, q // 4
        x20[32 * c : 32 * c + 20, 512 * qq : 512 * (qq + 1)] = \
            x20f[:, 512 * q : 512 * (q + 1)]
    return {"xwp": xwp, "x20": x20}


def _run(inputs, trace=False):
    from concourse.bass_utils import run_bass_kernel_spmd

    nc = _get_program()
    shared = _shared_inputs(
        inputs["r_nuclei"], inputs["charges"], inputs["spin_mask_parallel"],
        inputs["b_en"], inputs["b_ee"],
        inputs["W1_en"], inputs["b1_en"], inputs["W2_en"], inputs["b2_en"],
        inputs["W3_en"], inputs["b3_en"],
        inputs["W1_ee"], inputs["b1_ee"], inputs["W2_ee"], inputs["b2_ee"],
        inputs["W3_ee"], inputs["b3_ee"],
        inputs["scale_en"], inputs["scale_ee"],
    )
    r_el = np.asarray(inputs["r_electrons"], np.float32)
    in_maps = []
    for c in range(N_CORES):
        m = dict(shared)
        m.update(_core_inputs(r_el[c * WC : (c + 1) * WC]))
        in_maps.append(m)
    res = run_bass_kernel_spmd(nc, in_maps, list(range(N_CORES)), trace=trace)
    out = np.concatenate(
        [np.asarray(r["out"]).reshape(-1) for r in res.results]
    ).astype(np.float32)
    return out, res


def kernel(**inputs):
    out, _ = _run(inputs, trace=False)
    return out

